# revision 29
# baseline (speedup 1.0000x reference)
"""Sliding-window GQA attention (B=2,T=2048,D=2048,N=8,K=4,H=256,W=1024) on 8 trn2 cores.

Sharding: batch over 2 (fsdp) x heads over 4 (tp). Core (b, tp) computes 2 q heads /
1 kv head for batch b; partial [T, D] outputs are summed over tp on the host.

Per-core pipeline, all heavy matmuls in bf16 (1 cyc/row, fp32 PSUM; host pre-casts
and pre-shuffles operands so every DMA is a single contiguous descriptor):
  A: per 512-token quarter: qT/kT = W^T x^T (head-dim on partitions) and v (natural
     layout). All projection matmuls for the quarter issue back-to-back; the
     dependent sum-of-squares (ones-matmul, f32r) + rms (ln/exp on ACT, no rsqrt
     table) + RoPE chains trail one projection behind so the PE never head-of-line
     blocks on ACT/DVE.
  B: per 256-token query pair: logits^T = kT^T qT per 128-key block (window blocks
     only), exp on ACT straight to bf16, triangular edge masks on DVE/GpSimd,
     denominator via ones-matmul, 1/den = exp(-ln(den)) (same ACT table set as exp
     => zero ACT_TABLE_LOADs in phase B), P^T V via PE accumulation.
  C: out = pvT^T o_w, lagged one pair behind B to fill exp-wait windows.
Single TileContext + one shared 8-bank PSUM pool (tags pk/pq/vps reused across
phases) so there is no all-engine barrier between phases.
DMA rings: SP carries x^T stream + cos/sin + out stores; GpSimd ring carries all
weights/masks; ACT queue does no DMA triggering at all.
"""
import os

import numpy as np
import ml_dtypes

import concourse.bacc as bacc
import concourse.mybir as mybir
from concourse.tile import TileContext
from concourse.bass_utils import run_bass_kernel_spmd

try:  # pragma: no cover - profiling hook is optional
    from antenv.axon_hooks import get_axon_ntff_profile_hook  # noqa: F401
except ImportError:
    os.environ.setdefault("BASS_NEVER_TRACE", "1")


from concourse.hw_specs import get_activation_tables as _orig_act_tables


def _act_tables_ln_exp_coalesced(arch):
    """View of the activation-table list where Exp/Ln resolve only to the
    'natural_log_exp_and_others' set, so the table-load insertion pass keeps
    one table loaded instead of alternating between the exp-first and
    ln-first sets on every softmax/rms step. Order (and thus set ids) is
    unchanged; set 6 genuinely contains exp+ln so codegen is unaffected."""
    hide = {mybir.ActivationFunctionType.Exp, mybir.ActivationFunctionType.Ln}
    out = {}
    for name, funcs in _orig_act_tables(arch).items():
        if name != "natural_log_exp_and_others":
            funcs = funcs - hide
        out[name] = funcs
    return out


bacc.get_activation_tables = _act_tables_ln_exp_coalesced

F32 = mybir.dt.float32
F32R = mybir.dt.float32r
BF16 = mybir.dt.bfloat16
AF = mybir.ActivationFunctionType
OP = mybir.AluOpType

B, T, D = 2, 2048, 2048
N, KV, H = 8, 4, 256
WINDOW = 1024
BASE_FREQ = 10000.0
EPS = 1e-6
NB = T // 128          # 16 token blocks
NQ = 4                 # t quarters for projections (512 each)
NPAIR = 8              # query-block pairs (256 tokens each)
NPY_BF16 = ml_dtypes.bfloat16


def _mask_idx(i, j):
    if j == i + 1:
        return 3
    if j == i:
        return 2
    if j == i - 7:
        return 1
    if j == i - 8:
        return 0
    return None


def _jlist(i):
    return list(range(max(0, i - 8), i + 2))


def _build():
    nc = bacc.Bacc(None)

    # host-pre-shuffled layouts: partition dim first everywhere.
    xT = nc.dram_tensor("xT", [128, 16, T], BF16, kind="ExternalInput")
    qw = nc.dram_tensor("qw", [128, 2, 16, H], BF16, kind="ExternalInput")
    kw = nc.dram_tensor("kw", [128, 16, H], BF16, kind="ExternalInput")
    vw = nc.dram_tensor("vw", [128, 16, H], BF16, kind="ExternalInput")
    ow = nc.dram_tensor("ow", [128, 2, 2, D], BF16, kind="ExternalInput")
    cosT = nc.dram_tensor("cosT", [128, T], F32, kind="ExternalInput")
    sinT = nc.dram_tensor("sinT", [128, T], F32, kind="ExternalInput")
    masks = nc.dram_tensor("masks", [128, 4, 256], BF16, kind="ExternalInput")
    scs = nc.dram_tensor("scs", [128, 2, 2], F32, kind="ExternalInput")  # (1+scale)[q/k][hh]
    out = nc.dram_tensor("out", [T, D], BF16, kind="ExternalOutput")

    with TileContext(nc) as tc:
        with tc.tile_pool(name="pers", bufs=1) as pers, \
             tc.tile_pool(name="wts", bufs=1) as wts, \
             tc.tile_pool(name="xs", bufs=1) as xs, \
             tc.tile_pool(name="ropep", bufs=1) as ropep, \
             tc.tile_pool(name="expt", bufs=12) as expt, \
             tc.tile_pool(name="bw", bufs=2) as bw, \
             tc.tile_pool(name="oc", bufs=3) as oc, \
             tc.tile_pool(name="ps", bufs=1, space="PSUM") as psum:

            kT_sb = pers.tile([128, 2, T], BF16)
            v_sb = pers.tile([128, NB, H], BF16)
            qT_sb = pers.tile([128, 2, 2, T], BF16)
            pvT_sb = pers.tile([128, 2, 2, T], BF16)
            ow_sb = pers.tile([128, 2, 2, D], BF16)
            masks_sb = pers.tile([128, 4, 256], BF16)
            scs_sb = pers.tile([128, 2, 2], F32)
            ones32 = pers.tile([128, 128], F32)
            ones = pers.tile([128, 128], F32R)
            ones_b = pers.tile([128, 128], BF16)
            bias_q = pers.tile([128, 1], F32)
            bias_k = pers.tile([128, 1], F32)

            # weight-side DMAs ride the ACT HWDGE ring; first k-weight chunk
            # leads so the very first matmul can start ASAP.
            kw_sb = wts.tile([128, 16, H], BF16)
            vw_sb = wts.tile([128, 16, H], BF16)
            qw_sb = wts.tile([128, 2, 16, H], BF16)
            nc.scalar.dma_start(out=kw_sb[:, 0:2, :], in_=kw[:, 0:2, :])
            nc.scalar.dma_start(out=kw_sb[:, 2:6, :], in_=kw[:, 2:6, :])
            nc.scalar.dma_start(out=scs_sb, in_=scs[:, :, :])
            nc.scalar.dma_start(out=kw_sb[:, 6:16, :], in_=kw[:, 6:16, :])
            nc.scalar.dma_start(out=vw_sb[:, 0:8, :], in_=vw[:, 0:8, :])
            nc.scalar.dma_start(out=vw_sb[:, 8:16, :], in_=vw[:, 8:16, :])
            nc.scalar.dma_start(out=masks_sb, in_=masks[:, :, :])
            for nl in range(2):
                nc.scalar.dma_start(out=qw_sb[:, nl, 0:8, :], in_=qw[:, nl, 0:8, :])
                nc.scalar.dma_start(out=qw_sb[:, nl, 8:16, :], in_=qw[:, nl, 8:16, :])
            for nl in range(2):
                for hh in range(2):
                    nc.scalar.dma_start(out=ow_sb[:, nl, hh, :], in_=ow[:, nl, hh, :])

            nc.vector.memset(ones32, 1.0)
            nc.vector.memset(ones_b, 1.0)
            nc.vector.tensor_copy(ones, ones32)
            nc.vector.memset(bias_q, float(H * EPS))
            nc.vector.memset(bias_k, EPS)

            # PE warm-up: dummy matmuls while the first weight/x DMAs land so
            # the clock is fully ramped (p-state) when real work arrives.
            pwarm = psum.tile([128, 128], F32, tag="pq", bufs=4, name="pwarm")
            for w in range(24):
                nc.tensor.matmul(pwarm, ones_b, ones_b,
                                 start=(w == 0), stop=(w == 23))

            # ---------------- Phase A: projections + rms + rope ----------------
            def rope_sq(p0, p1):
                # squares issued immediately after the projection's stop-matmul
                sq0 = ropep.tile([128, 512], F32R, tag="sq0", bufs=2, name="sq0")
                sq1 = ropep.tile([128, 512], F32R, tag="sq1", bufs=2, name="sq1")
                nc.scalar.activation(sq0, p0, AF.Square)
                nc.scalar.activation(sq1, p1, AF.Square)
                return sq0, sq1

            def rope_cp(p0, p1, kind):
                # PSUM->SBUF drain of the raw projection, with (1+scale) folded
                # in via ACT scale; issued right after the squares so the bank
                # frees early (GpSimd cannot read PSUM either way).
                ki = 0 if kind == "q" else 1
                s0 = scs_sb[:, ki, 0:1]
                s1 = scs_sb[:, ki, 1:2]
                c0 = ropep.tile([128, 512], F32, tag="c0", bufs=2, name="c0")
                c1 = ropep.tile([128, 512], F32, tag="c1", bufs=2, name="c1")
                nc.scalar.activation(c0, p0, AF.Copy, scale=s0)
                nc.scalar.activation(c1, p1, AF.Copy, scale=s1)
                return c0, c1

            def rope_fin(c0, c1, sq0, sq1, dst, kind, cs_t, ss_t):
                # pss matmul + rms via exp(-ln(ss+eps)/2) + rope; issued under
                # cover of later independent matmuls.
                pss = psum.tile([128, 512], F32, tag="vps", bufs=2, name="pss")
                nc.tensor.matmul(pss, ones, sq0, start=True, stop=False)
                nc.tensor.matmul(pss, ones, sq1, start=False, stop=True)
                tln = ropep.tile([128, 512], F32, tag="tln", bufs=2, name="tln")
                rs = ropep.tile([128, 512], F32, tag="rs", bufs=2, name="rs")
                if kind == "q":
                    # 1/16 * rsqrt(ss/256 + eps) == 1/sqrt(ss + 256*eps)
                    nc.scalar.activation(tln, pss, AF.Ln, scale=1.0, bias=bias_q)
                else:
                    nc.scalar.activation(tln, pss, AF.Ln, scale=1.0 / H, bias=bias_k)
                nc.scalar.activation(rs, tln, AF.Exp, scale=-0.5)
                cs = ropep.tile([128, 512], F32, tag="cs", bufs=2, name="cs")
                ss = ropep.tile([128, 512], F32, tag="ss", bufs=2, name="ss")
                nc.vector.tensor_tensor(cs, cs_t, rs, OP.mult)
                nc.vector.tensor_tensor(ss, ss_t, rs, OP.mult)
                t0 = ropep.tile([128, 512], F32, tag="t0", bufs=2, name="t0")
                t1 = ropep.tile([128, 512], F32, tag="t1", bufs=2, name="t1")
                nc.vector.tensor_tensor(t0, c0, cs, OP.mult)
                nc.vector.tensor_tensor(t1, c1, ss, OP.mult)
                nc.vector.tensor_tensor(dst[:, 0, :], t0, t1, OP.subtract)
                # second half on GpSimd so the last chain's tail is short
                t2 = ropep.tile([128, 512], F32, tag="t2", bufs=2, name="t2")
                t3 = ropep.tile([128, 512], F32, tag="t3", bufs=2, name="t3")
                nc.gpsimd.tensor_tensor(t2, c1, cs, OP.mult)
                nc.gpsimd.tensor_tensor(t3, c0, ss, OP.mult)
                nc.gpsimd.tensor_tensor(dst[:, 1, :], t2, t3, OP.add)

            def emit_quarter(qt):
                tq = slice(512 * qt, 512 * (qt + 1))
                xts = []
                for c in range(8):
                    xt = xs.tile([128, 2, 512], BF16, tag="xt", bufs=10, name="xt")
                    if qt == 0 and c == 0:
                        # split so the very first matmul waits on half the bytes
                        nc.sync.dma_start(out=xt[:, 0, :], in_=xT[:, 0, tq])
                        nc.sync.dma_start(out=xt[:, 1, :], in_=xT[:, 1, tq])
                    else:
                        nc.sync.dma_start(out=xt, in_=xT[:, 2 * c:2 * c + 2, tq])
                    xts.append(xt)

                def xsl(d, tl=slice(0, 512)):
                    return xts[d // 2][:, d % 2, tl]

                cs_t = ropep.tile([128, 512], F32, tag="cst", bufs=2, name="cst")
                ss_t = ropep.tile([128, 512], F32, tag="sst", bufs=2, name="sst")
                nc.sync.dma_start(out=cs_t, in_=cosT[:, tq])
                nc.sync.dma_start(out=ss_t, in_=sinT[:, tq])

                # k projection (h-halves in separate banks, interleaved per d)
                pk = [psum.tile([128, 512], F32, tag="pk", bufs=2, name=f"pk{qt}_{hh}")
                      for hh in range(2)]
                for d in range(16):
                    nc.tensor.matmul(pk[0], kw_sb[:, d, 0:128], xsl(d),
                                     start=(d == 0), stop=(d == 15))
                    nc.tensor.matmul(pk[1], kw_sb[:, d, 128:256], xsl(d),
                                     start=(d == 0), stop=(d == 15))
                sqk = rope_sq(pk[0], pk[1])
                cpk = rope_cp(pk[0], pk[1], "k")

                def emit_v():
                    for half in range(2):
                        p = psum.tile([128, 2, H], F32, tag="vps", bufs=2, name=f"pv{qt}_{half}")
                        tc0 = 4 * qt + 2 * half
                        for sub in range(2):
                            tl = slice(128 * (2 * half + sub), 128 * (2 * half + sub) + 128)
                            for d in range(16):
                                nc.tensor.matmul(p[:, sub, :], xsl(d, tl), vw_sb[:, d, :],
                                                 start=(d == 0), stop=(d == 15))
                        nc.scalar.copy(v_sb[:, tc0:tc0 + 2, :], p)

                # v (natural layout); in the last quarter the v matmuls move
                # between the q rope tails so they cover the trailing ACT
                # chain right before phase B's first exps.
                last = qt == NQ - 1
                if not last:
                    emit_v()

                # q heads; rope_fin(k) is issued between q0 and q1 so its pss
                # matmul never reaches the PE queue head before its sq is done.
                sqq = {}
                cpq = {}
                for nl in range(2):
                    pq_ = {}
                    for hh in range(2):
                        pq_[hh] = psum.tile([128, 512], F32, tag="pq", bufs=4,
                                            name=f"pq{qt}_{nl}_{hh}")
                        hs = slice(128 * hh, 128 * (hh + 1))
                        for d in range(16):
                            nc.tensor.matmul(pq_[hh], qw_sb[:, nl, d, hs], xsl(d),
                                             start=(d == 0), stop=(d == 15))
                    sqq[nl] = rope_sq(pq_[0], pq_[1])
                    cpq[nl] = rope_cp(pq_[0], pq_[1], "q")
                    if nl == 0:
                        rope_fin(*cpk, *sqk, kT_sb[:, :, tq], "k", cs_t, ss_t)
                rope_fin(*cpq[0], *sqq[0], qT_sb[:, 0, :, tq], "q", cs_t, ss_t)
                if last:
                    emit_v()
                    # defer the last q-head rope chain: the caller emits it
                    # after the first B pairs' logits, off the critical path
                    return lambda: rope_fin(*cpq[1], *sqq[1],
                                            qT_sb[:, 1, :, tq], "q", cs_t, ss_t)
                rope_fin(*cpq[1], *sqq[1], qT_sb[:, 1, :, tq], "q", cs_t, ss_t)

            # ---------------- Phases B + C ----------------
            def emit_logits_exp(pi):
                i = 2 * pi
                tqs = slice(256 * pi, 256 * (pi + 1))
                js = _jlist(i)
                ets = {}
                for nl in range(2):
                    for k in range(0, len(js), 2):
                        jp = js[k:k + 2]
                        lp = psum.tile([128, 2, 256], F32, tag="pq", bufs=4, name="lp")
                        for x2, j in enumerate(jp):
                            sj = slice(128 * j, 128 * (j + 1))
                            nc.tensor.matmul(lp[:, x2, :], kT_sb[:, 0, sj],
                                             qT_sb[:, nl, 0, tqs],
                                             start=True, stop=False)
                            nc.tensor.matmul(lp[:, x2, :], kT_sb[:, 1, sj],
                                             qT_sb[:, nl, 1, tqs],
                                             start=False, stop=True)
                        et = expt.tile([128, 2, 256], BF16, tag="et", name="et")
                        nc.scalar.activation(et, lp, AF.Exp)
                        for x2, j in enumerate(jp):
                            mi = _mask_idx(i, j)
                            if mi is not None:
                                eng = nc.vector if (j % 2 == 0) else nc.gpsimd
                                eng.tensor_tensor(et[:, x2, :], et[:, x2, :],
                                                  masks_sb[:, mi, :], OP.mult)
                            ets[(nl, j)] = et[:, x2, :]
                return ets

            def emit_tail(pi, ets):
                i = 2 * pi
                tqs = slice(256 * pi, 256 * (pi + 1))
                js = _jlist(i)
                for nl in range(2):
                    pd = psum.tile([128, 256], F32, tag="vps", bufs=2, name="pd")
                    for idx, j in enumerate(js):
                        nc.tensor.matmul(pd, ones_b, ets[(nl, j)],
                                         start=(idx == 0), stop=(idx == len(js) - 1))
                    # 1/den = exp(-ln(den)): stays in the exp table set, no swaps
                    tl2 = bw.tile([128, 256], F32, tag="tl2", name="tl2")
                    r2 = bw.tile([128, 256], F32, tag="r2", name="r2")
                    nc.scalar.activation(tl2, pd, AF.Ln)
                    nc.scalar.activation(r2, tl2, AF.Exp, scale=-1.0)
                    for hh in range(2):
                        pv = psum.tile([128, 256], F32, tag="pk", bufs=2, name="pvb")
                        hs = slice(128 * hh, 128 * (hh + 1))
                        for idx, j in enumerate(js):
                            nc.tensor.matmul(pv, v_sb[:, j, hs], ets[(nl, j)],
                                             start=(idx == 0), stop=(idx == len(js) - 1))
                        nc.vector.tensor_tensor(pvT_sb[:, nl, hh, tqs], pv, r2, OP.mult)

            def emit_oproj(pi):
                for tb in (2 * pi, 2 * pi + 1):
                    ts_ = slice(128 * tb, 128 * (tb + 1))
                    for dh in range(2):
                        od = oc.tile([128, 1024], BF16, tag="od", bufs=3, name="od")
                        for dx in range(2):
                            dt = 2 * dh + dx
                            dsl = slice(512 * dt, 512 * (dt + 1))
                            po = psum.tile([128, 512], F32, tag="pq", bufs=4, name="po")
                            step = 0
                            for nl in range(2):
                                for hh in range(2):
                                    nc.tensor.matmul(po, pvT_sb[:, nl, hh, ts_],
                                                     ow_sb[:, nl, hh, dsl],
                                                     start=(step == 0), stop=(step == 3))
                                    step += 1
                            if dx == 0:
                                nc.vector.tensor_copy(od[:, 0:512], po)
                            else:
                                nc.scalar.copy(od[:, 512:1024], po)
                        nc.sync.dma_start(out=out[ts_, 1024 * dh:1024 * (dh + 1)], in_=od)

            for qt in range(NQ):
                fin_last = emit_quarter(qt)
            # B: the first two pairs pipeline logits ahead of tails so the
            # phase-A ACT/DVE backlog is covered by matmul work; after that,
            # tail-follows-logits with o-proj lagging to fill exp waits.
            ets0 = emit_logits_exp(0)
            ets1 = emit_logits_exp(1)
            fin_last()
            emit_tail(0, ets0)
            emit_tail(1, ets1)
            for pi in range(2, NPAIR):
                emit_tail(pi, emit_logits_exp(pi))
                emit_oproj(pi - 2)
            emit_oproj(NPAIR - 2)
            emit_oproj(NPAIR - 1)

    nc.compile()
    return nc


_prog = None
last_results = None


def kernel(x, positions, q_w, k_w, v_w, o_w, q_norm_scale, k_norm_scale):
    global _prog, last_results
    x = np.asarray(x); positions = np.asarray(positions)
    q_w = np.asarray(q_w); k_w = np.asarray(k_w); v_w = np.asarray(v_w); o_w = np.asarray(o_w)
    q_norm_scale = np.asarray(q_norm_scale); k_norm_scale = np.asarray(k_norm_scale)

    if _prog is None:
        _prog = _build()
    nc = _prog

    # host-side constants
    j = np.arange(H // 2, dtype=np.float32)
    timescale = (BASE_FREQ ** (2.0 / H * j)).astype(np.float32)

    c = np.arange(128)[:, None]
    r = np.arange(128)[None, :]
    up = (c <= r).astype(np.float32)
    lo = (c > r).astype(np.float32)
    one_b = np.ones((128, 128), np.float32)
    zero_b = np.zeros((128, 128), np.float32)
    masks_np = np.stack([
        np.concatenate([lo, zero_b], 1),
        np.concatenate([one_b, lo], 1),
        np.concatenate([up, one_b], 1),
        np.concatenate([zero_b, up], 1),
    ], axis=0).transpose(1, 0, 2).astype(NPY_BF16)  # [128, 4, 256]

    scs_np = np.empty((128, 2, 2), np.float32)
    scs_np[:, 0, 0] = 1.0 + q_norm_scale[:128]
    scs_np[:, 0, 1] = 1.0 + q_norm_scale[128:]
    scs_np[:, 1, 0] = 1.0 + k_norm_scale[:128]
    scs_np[:, 1, 1] = 1.0 + k_norm_scale[128:]

    def shuf_w(w):  # [D, H] -> [128, 16, H]
        return np.ascontiguousarray(w.reshape(16, 128, H).transpose(1, 0, 2)).astype(NPY_BF16)

    in_maps = []
    for core in range(8):
        b, tp = core // 4, core % 4
        sinu = positions[b].astype(np.float32)[:, None] / timescale[None, :]  # [T, 128]
        xt_full = x[b].T  # [D, T]
        in_maps.append({
            "xT": np.ascontiguousarray(
                xt_full.reshape(16, 128, T).transpose(1, 0, 2)).astype(NPY_BF16),
            "qw": np.ascontiguousarray(
                q_w[2 * tp: 2 * tp + 2].reshape(2, 16, 128, H).transpose(2, 0, 1, 3)
            ).astype(NPY_BF16),
            "kw": shuf_w(k_w[tp]),
            "vw": shuf_w(v_w[tp]),
            "ow": np.ascontiguousarray(
                o_w[2 * tp: 2 * tp + 2].reshape(2, 2, 128, D).transpose(2, 0, 1, 3)
            ).astype(NPY_BF16),
            "cosT": np.ascontiguousarray(np.cos(sinu).T).astype(np.float32),
            "sinT": np.ascontiguousarray(np.sin(sinu).T).astype(np.float32),
            "masks": masks_np,
            "scs": scs_np,
        })

    res = run_bass_kernel_spmd(nc, in_maps, core_ids=list(range(8)))
    last_results = res

    out = np.zeros((B, T, D), np.float32)
    for core in range(8):
        out[core // 4] += res.results[core]["out"].astype(np.float32)
    return out
